# revision 25
# baseline (speedup 1.0000x reference)
"""Causal multi-head attention (B=2, T=2048, C=1024, H=16) on 8 trn2 NeuronCores.

Sharding: core = (b, g): b = core // 4 (batch), g = core % 4 (head group of 4
heads).  Each core:
  xp^T = x[b]^T + pe^T                                  [C, T]
  qk^T = w_qk_local^T.T @ xp^T                          [512, T]  (q,k of 4 heads, transposed)
  v    = xp^T.T @ w_v_local^T                           [T, 256]  (natural layout, + ones col)
  per head h, per 512-wide query block: scores^T = k_h^T.T @ q_h^T (causal,
  lower j-tiles only), p^T = exp(scores^T / 8 - 3) (masked diag),
  y^T = [v|1]^T.T @ p^T accumulated over j-tiles -> row 64 of y^T is the
  softmax denominator (the -3 bias cancels in the ratio; it keeps f16 safe).
  y_cat^T[c_local, t] = y^T / denom; out_partial = y_cat^T.T @ w_proj_local^T  [T, C]
Host sums the 4 partial outputs per batch (unshard of the row-sharded proj).

All matmuls run in float16 (full-rate PE mode, fp32 PSUM accumulation).
"""

import numpy as np

B, T, C, H = 2, 2048, 1024, 16
NCORES = 8
GROUPS = 4            # head-groups across cores (tensor parallel)
HL = H // GROUPS      # heads per core = 4
D = C // H            # 64
CL = HL * D           # 256 local channels
KC = C // 128         # 8 contraction tiles over C
JG = 2                # j-tiles per scores psum tile (exp batch)
N_WARM = 14           # PE warmup matmuls (HAM un-throttle during initial DMA)

_PROG_CACHE = {}


def _build_program(t_len=T, debug_taps=False):
    from contextlib import ExitStack

    import concourse.tile as tile
    from concourse import bacc, mybir
    from concourse.masks import make_upper_triangular

    f32 = mybir.dt.float32
    f16 = mybir.dt.float16

    nt = t_len // 512     # 512-wide t chunks
    mt_n = t_len // 128   # 128-wide t tiles

    nc = bacc.Bacc("TRN2", target_bir_lowering=False, debug=False,
                   num_devices=NCORES)

    # chunk-major, partition-tiled host layouts -> fully contiguous DMAs
    x_r = nc.dram_tensor("x_t", [t_len // 512, 128, KC, 512], f16,
                         kind="ExternalInput").ap()
    pe_r = nc.dram_tensor("pe_t", [t_len // 512, 128, KC, 512], f16,
                          kind="ExternalInput").ap()
    wqk_r = nc.dram_tensor("w_qk_t", [128, KC, 2 * CL], f16,
                           kind="ExternalInput").ap()
    wv_r = nc.dram_tensor("w_v_t", [128, KC, CL], f16,
                          kind="ExternalInput").ap()
    wproj_r = nc.dram_tensor("w_proj_t", [128, CL // 128, C], f16,
                             kind="ExternalInput").ap()
    out = nc.dram_tensor("out", [t_len, C], f32, kind="ExternalOutput").ap()

    dbg = {}
    if debug_taps:
        nb_ = t_len // 512
        dbg["qk"] = nc.dram_tensor("dbg_qk", [4, 128, t_len], f16,
                                   kind="ExternalOutput").ap()
        dbg["v"] = nc.dram_tensor("dbg_v", [mt_n, 128, HL * (D + 1)], f16,
                                  kind="ExternalOutput").ap()
        dbg["ycat"] = nc.dram_tensor("dbg_ycat", [CL // 128, 128, t_len], f16,
                                     kind="ExternalOutput").ap()
        dbg["sum"] = nc.dram_tensor("dbg_sum", [nb_, HL, 512], f32,
                                    kind="ExternalOutput").ap()
        dbg["recip"] = nc.dram_tensor("dbg_recip", [nb_, HL, 512], f32,
                                      kind="ExternalOutput").ap()
        dbg["rb"] = nc.dram_tensor("dbg_rb", [nb_, HL, 64, 512], f32,
                                   kind="ExternalOutput").ap()

    with tile.TileContext(nc) as tc:
        with ExitStack() as ctx:
            const_p = ctx.enter_context(tc.tile_pool(name="const", bufs=1))
            w_p = ctx.enter_context(tc.tile_pool(name="weights", bufs=1))
            act_p = ctx.enter_context(tc.tile_pool(name="acts", bufs=1))
            work_p = ctx.enter_context(tc.tile_pool(name="work", bufs=2))
            ps_p = ctx.enter_context(tc.tile_pool(name="ps", bufs=2, space="PSUM"))

            mask_ut = const_p.tile([128, 128], f16, tag="mask")
            make_upper_triangular(nc, mask_ut[:], val=1.0, diag=True)
            ones_f32 = const_p.tile([128, HL], f32, tag="ones")
            nc.vector.memset(ones_f32[:], 1.0)
            expbias = const_p.tile([128, 1], f32, tag="expbias")
            nc.vector.memset(expbias[:], -3.0)
            wrm = const_p.tile([128, 512], f16, tag="wrm")
            nc.vector.memset(wrm[:], 0.0)

            # PE warmup: dense dummy f16 matmuls while the first DMAs land,
            # so the HAM clock gate is at 8/8 when real matmuls start.
            for _ in range(N_WARM):
                wps = ps_p.tile([128, 512], f32, tag="mmps", name="wps")
                nc.tensor.matmul(wps[:], lhsT=wrm[:, 0:128], rhs=wrm[:],
                                 start=True, stop=True)

            wqk_sb = w_p.tile([128, KC, 2 * CL], f16, tag="wqk")
            wv_sb = w_p.tile([128, KC, CL], f16, tag="wv")
            wproj_sb = w_p.tile([128, CL // 128, C], f16, tag="wpj")

            # ---- persistent activations ----
            # qk^T rows: m0 = q heads 0,1; m1 = q heads 2,3; m2 = k h0,1; m3 = k h2,3
            qk_sb = [act_p.tile([128, t_len], f16, tag=f"qk{m}", name="qk")
                     for m in range(4)]
            # v tiles, per 128-t tile: 4 heads x (64 v cols + ones col)
            v_sb = [act_p.tile([128, HL * (D + 1)], f16, tag=f"v{m}", name="v")
                    for m in range(mt_n)]
            for m in range(mt_n):
                ones_col = v_sb[m].rearrange("p (h e) -> p h e", e=D + 1)[:, :, D:D + 1]
                nc.vector.tensor_copy(ones_col,
                                      ones_f32.rearrange("p (h o) -> p h o", o=1))
            # y_cat^T [256, T] as 2 tiles of 128 partitions
            ycat_sb = [act_p.tile([128, t_len], f16, tag=f"ycat{k}", name="ycat")
                       for k in range(CL // 128)]

            xp_of = {}

            def emit_chunk_dma(n):
                """x/pe of chunk n in 4 quarter-DMAs each (parallel HWDGE
                queues), with the add emitted per quarter for pipelining."""
                x_s = work_p.tile([128, KC, 512], f16, tag="x", bufs=1, name="x_s")
                pe_s = work_p.tile([128, KC, 512], f16, tag="pe", bufs=1,
                                   name="pe_s")
                xp = work_p.tile([128, KC, 512], f16, tag="xp", name="xp")
                for q in range(4):
                    sl = slice(2 * q, 2 * q + 2)
                    nc.sync.dma_start(out=x_s[:, sl, :], in_=x_r[n, :, sl, :])
                    nc.scalar.dma_start(out=pe_s[:, sl, :], in_=pe_r[n, :, sl, :])
                    nc.vector.tensor_add(xp[:, sl, :], x_s[:, sl, :],
                                         pe_s[:, sl, :])
                xp_of[n] = xp

            def chunk_fillers(n):
                """qk / v m-tile units for chunk n, as weavable closures."""
                def qk_unit(i):
                    def go():
                        xp = xp_of[n]
                        ps = ps_p.tile([128, 512], f32, tag="mmps", name="qk_ps")
                        for ck in range(KC):
                            nc.tensor.matmul(
                                ps[:],
                                lhsT=wqk_sb[:, ck, i * 128:(i + 1) * 128],
                                rhs=xp[:, ck, :],
                                start=(ck == 0), stop=(ck == KC - 1))
                        nc.vector.tensor_copy(
                            qk_sb[i][:, n * 512:(n + 1) * 512], ps[:])
                    return go

                def v_unit(i):
                    def go():
                        xp = xp_of[n]
                        psv = ps_p.tile([128, CL], f32, tag="mmps", name="v_ps")
                        for ck in range(KC):
                            nc.tensor.matmul(
                                psv[:],
                                lhsT=xp[:, ck, i * 128:(i + 1) * 128],
                                rhs=wv_sb[:, ck, :],
                                start=(ck == 0), stop=(ck == KC - 1))
                        mt = 4 * n + i
                        nc.vector.tensor_copy(
                            v_sb[mt].rearrange("p (h e) -> p h e",
                                               e=D + 1)[:, :, 0:D],
                            psv.rearrange("p (h e) -> p h e", e=D))
                    return go
                units = []
                for i in range(4):
                    units.append(qk_unit(i))
                    units.append(v_unit(i))
                return units

            def emit_chunk_compute(n):
                for f in chunk_fillers(n):
                    f()

            def _emit_norm_head(c, h, recips4, ysbs):
                hb = (h % 2) * 64
                rbsrc = work_p.tile([1, 512], f32, tag="rbsrc", bufs=4,
                                    name="rbsrc")
                nc.vector.tensor_copy(rbsrc[:], recips4[32 * h:32 * h + 1, :])
                rb = work_p.tile([64, 512], f32, tag="rb", bufs=3, name="rb")
                nc.gpsimd.partition_broadcast(rb[:], rbsrc[:])
                if dbg:
                    nc.sync.dma_start(out=dbg["rb"][c, h], in_=rb[:])
                nc.vector.tensor_mul(
                    ycat_sb[h // 2][hb:hb + 64, c * 512:(c + 1) * 512],
                    ysbs[h][0:64, :], rb[:])

            def emit_attn_block(c, fillers=(), split_tail=False):
                """Attention for query block i in [512c, 512c+512), all heads.
                One filler (qk/v/proj m-tile) is woven in after each score/AV
                group so the in-order PE queue has work during exp latency."""
                fillers = list(fillers)
                njt = 4 * c + 4
                sums4 = work_p.tile([128, 512], f32, tag="sums4", name="sums4")
                nc.vector.memset(sums4[:], 1.0)
                ysbs = {}
                # heads processed in interleaved pairs: head h+1's scores fill
                # the PE while head h's exp runs (in-order engine queues)
                for hp in (0, 2):
                    yps_of, esb_of, offs_of = {}, {}, {}
                    for h in (hp, hp + 1):
                        yps_of[h] = ps_p.tile([65, 512], f32, tag="yps",
                                              name="yps")
                    for g0 in range(0, njt, JG):
                        jts = range(g0, min(g0 + JG, njt))
                        for h in (hp, hp + 1):
                            hb = (h % 2) * 64
                            q_tile = qk_sb[h // 2]
                            k_tile = qk_sb[2 + h // 2]
                            sps = ps_p.tile([128, JG * 512], f32, tag="sps",
                                            name="sps")
                            esb = work_p.tile([128, JG * 512], f16, tag="esb",
                                              bufs=4, name="esb")
                            offs = {}
                            cover_end = None
                            for jt in jts:
                                off = max(0, (jt - 4 * c)) * 128
                                offs[jt] = off
                                ls = (jt - g0) * 512 + off
                                width = 512 - off
                                if cover_end is not None and ls > cover_end:
                                    # dead gap between j-tile ranges: zero it so
                                    # the batched exp reads initialized psum
                                    nc.vector.memset(sps[:, cover_end:ls], 0.0)
                                cover_end = ls + width
                                nc.tensor.matmul(
                                    sps[:, ls:ls + width],
                                    lhsT=k_tile[hb:hb + 64,
                                                jt * 128:(jt + 1) * 128],
                                    rhs=q_tile[hb:hb + 64,
                                               c * 512 + off:(c + 1) * 512],
                                    start=True, stop=True)
                            gfirst = offs[jts[0]]
                            gend = (jts[-1] - g0) * 512 + 512
                            nc.scalar.activation(
                                esb[:, gfirst:gend], sps[:, gfirst:gend],
                                mybir.ActivationFunctionType.Exp, scale=0.125,
                                bias=expbias[:])
                            for jt in jts:
                                if jt >= 4 * c:  # diagonal tile: causal mask
                                    ls = (jt - g0) * 512 + offs[jt]
                                    nc.vector.tensor_mul(
                                        esb[:, ls:ls + 128], esb[:, ls:ls + 128],
                                        mask_ut[:])
                            esb_of[h], offs_of[h] = esb, offs
                        for h in (hp, hp + 1):
                            esb, offs = esb_of[h], offs_of[h]
                            for jt in jts:
                                off = offs[jt]
                                ls = (jt - g0) * 512 + off
                                nc.tensor.matmul(
                                    yps_of[h][:, off:512],
                                    lhsT=v_sb[jt][:, h * (D + 1):
                                                  (h + 1) * (D + 1)],
                                    rhs=esb[:, ls:ls + (512 - off)],
                                    start=(jt == 0), stop=(jt == njt - 1))
                        if fillers:
                            fillers.pop(0)()
                    for h in (hp, hp + 1):
                        ysb = work_p.tile([65, 512], f32, tag="ysb", bufs=6,
                                          name="ysb")
                        nc.vector.tensor_copy(ysb[:], yps_of[h][:])
                        nc.vector.tensor_copy(sums4[32 * h:32 * h + 1, :],
                                              ysb[64:65, :])
                        ysbs[h] = ysb
                    if split_tail:
                        rc = work_p.tile([128, 512], f32, tag="recips4",
                                         name="rc")
                        ln_t = work_p.tile([128, 512], f32, tag="lns",
                                           name="ln_t")
                        nc.scalar.activation(ln_t[:], sums4[:],
                                             mybir.ActivationFunctionType.Ln)
                        nc.scalar.activation(rc[:], ln_t[:],
                                             mybir.ActivationFunctionType.Exp,
                                             scale=-1.0)
                        for h in (hp, hp + 1):
                            _emit_norm_head(c, h, rc, ysbs)
                if split_tail:
                    for f in fillers:
                        f()
                    del fillers[:]
                    return
                recips4 = work_p.tile([128, 512], f32, tag="recips4",
                                      name="recips4")
                lns = work_p.tile([128, 512], f32, tag="lns", name="lns")
                nc.scalar.activation(lns[:], sums4[:],
                                     mybir.ActivationFunctionType.Ln)
                nc.scalar.activation(recips4[:], lns[:],
                                     mybir.ActivationFunctionType.Exp,
                                     scale=-1.0)
                if dbg:
                    for hh in range(HL):
                        nc.sync.dma_start(out=dbg["sum"][c, hh],
                                          in_=sums4[32 * hh:32 * hh + 1, :])
                        nc.sync.dma_start(out=dbg["recip"][c, hh],
                                          in_=recips4[32 * hh:32 * hh + 1, :])
                for f in fillers:
                    f()
                del fillers[:]
                for h in range(HL):
                    _emit_norm_head(c, h, recips4, ysbs)

            def proj_fillers(c):
                def unit(i):
                    def go():
                        _emit_proj_mt(c, i)
                    return go
                return [unit(i) for i in range(4)]

            def emit_proj_block(c, fast_tail=False):
                """out rows [512c, 512c+512)."""
                for i in range(4):
                    _emit_proj_mt(c, i, fast_tail)

            def _emit_proj_mt(c, i, fast_tail=False):
                    mt = 4 * c + i
                    osb = work_p.tile([128, C], f32, tag="osb", bufs=3, name="osb")
                    for n2 in range(C // 512):
                        ps = ps_p.tile([128, 512], f32, tag="mmps", name="proj_ps")
                        for kk in range(CL // 128):
                            nc.tensor.matmul(
                                ps[:],
                                lhsT=ycat_sb[kk][:, mt * 128:(mt + 1) * 128],
                                rhs=wproj_sb[:, kk, n2 * 512:(n2 + 1) * 512],
                                start=(kk == 0), stop=(kk == CL // 128 - 1))
                        if fast_tail and n2 % 2 == 0:
                            nc.scalar.copy(osb[:, n2 * 512:(n2 + 1) * 512], ps[:])
                        else:
                            nc.vector.tensor_copy(
                                osb[:, n2 * 512:(n2 + 1) * 512], ps[:])
                    if fast_tail:
                        nc.sync.dma_start(out=out[mt * 128:(mt + 1) * 128, 0:512],
                                          in_=osb[:, 0:512])
                        nc.scalar.dma_start(
                            out=out[mt * 128:(mt + 1) * 128, 512:C],
                            in_=osb[:, 512:C])
                    else:
                        nc.sync.dma_start(out=out[mt * 128:(mt + 1) * 128, :],
                                          in_=osb[:])

            # attn(c) needs qkv chunks <= c; proj(c) needs attn(c).  Chunk
            # n+1's DMA + add are emitted before attn(n) (not queued behind the
            # attention tail), and qkv(n+1)/proj(n-1) m-tiles are woven into
            # attn(n)'s groups as PE fillers for the exp latency.
            emit_chunk_dma(0)
            # weights after chunk-0 x/pe so the first adds aren't starved;
            # wqk in 4 quarter-DMAs (parallel queues)
            for q in range(4):
                sl = slice(2 * q, 2 * q + 2)
                nc.scalar.dma_start(out=wqk_sb[:, sl, :], in_=wqk_r[:, sl, :])
            units0 = chunk_fillers(0)
            for u in units0[0::2]:      # qk units first (need only wqk)
                u()
            # wv / wproj DMAs gated behind a tiny memset so their transfers
            # don't steal HBM bandwidth from the startup-critical set
            nc.vector.memset(wv_sb[0:1, 0:1, 0:1], 0.0)
            nc.vector.memset(wproj_sb[0:1, 0:1, 0:1], 0.0)
            nc.scalar.dma_start(out=wv_sb[:], in_=wv_r[:])
            nc.scalar.dma_start(out=wproj_sb[:], in_=wproj_r[:])
            for u in units0[1::2]:      # v units after
                u()
            for n in range(1, nt):
                emit_chunk_dma(n)
                emit_attn_block(n - 1)
                emit_chunk_compute(n)
                if n >= 2:
                    emit_proj_block(n - 2)
            emit_attn_block(nt - 1, split_tail=True)
            if nt >= 2:
                emit_proj_block(nt - 2)
            emit_proj_block(nt - 1, fast_tail=True)

            if dbg:
                for m in range(4):
                    nc.sync.dma_start(out=dbg["qk"][m], in_=qk_sb[m][:])
                for m in range(mt_n):
                    nc.sync.dma_start(out=dbg["v"][m], in_=v_sb[m][:])
                for k in range(CL // 128):
                    nc.sync.dma_start(out=dbg["ycat"][k], in_=ycat_sb[k][:])

    nc.compile()
    return nc


def _shard_inputs(x, w_qkv, w_proj, pe, t_len=T):
    x = np.asarray(x, dtype=np.float32).astype(np.float16)
    w_qkv = np.asarray(w_qkv, dtype=np.float32).astype(np.float16)
    w_proj = np.asarray(w_proj, dtype=np.float32).astype(np.float16)
    pe = np.asarray(pe, dtype=np.float32).astype(np.float16)

    def chunk_major(a_t):      # [C, t] -> [nt, 128, KC, 512]
        return np.ascontiguousarray(
            a_t.reshape(KC, 128, t_len // 512, 512).transpose(2, 1, 0, 3))

    def part_tiled(w_t):       # [C_in, M] -> [128, C_in//128, M]
        return np.ascontiguousarray(
            w_t.reshape(-1, 128, w_t.shape[1]).transpose(1, 0, 2))

    pe_t = chunk_major(pe[:t_len].T)
    x_ts = [chunk_major(x[b, :t_len].T) for b in range(x.shape[0])]
    in_maps = []
    for core in range(NCORES):
        b, g = core // GROUPS, core % GROUPS
        rows_q = w_qkv[g * CL:(g + 1) * CL]
        rows_k = w_qkv[C + g * CL:C + (g + 1) * CL]
        rows_v = w_qkv[2 * C + g * CL:2 * C + (g + 1) * CL]
        in_maps.append({
            "x_t": x_ts[b],
            "pe_t": pe_t,
            "w_qk_t": part_tiled(np.concatenate([rows_q, rows_k], axis=0).T.copy()),
            "w_v_t": part_tiled(rows_v.T.copy()),
            "w_proj_t": part_tiled(w_proj[:, g * CL:(g + 1) * CL].T.copy()),
        })
    return in_maps


_RUN_KWARGS = {}       # test-harness hook (e.g. trace=True); empty when graded
_LAST_RESULT = None


def kernel(x, w_qkv, w_proj, pe):
    global _LAST_RESULT
    from concourse import bass_utils

    if T not in _PROG_CACHE:
        _PROG_CACHE[T] = _build_program(T)
    nc = _PROG_CACHE[T]

    in_maps = _shard_inputs(x, w_qkv, w_proj, pe)
    res = bass_utils.run_bass_kernel_spmd(nc, in_maps, core_ids=list(range(NCORES)),
                                          **_RUN_KWARGS)
    _LAST_RESULT = res

    out = np.zeros((B, T, C), dtype=np.float32)
    for core in range(NCORES):
        out[core // GROUPS] += res.results[core]["out"]
    return out


# revision 26
# speedup vs baseline: 1.0588x; 1.0588x over previous
"""Causal multi-head attention (B=2, T=2048, C=1024, H=16) on 8 trn2 NeuronCores.

Sharding: core = (b, g): b = core // 4 (batch), g = core % 4 (head group of 4
heads).  Each core:
  xp^T = x[b]^T + pe^T                                  [C, T]
  qk^T = w_qk_local^T.T @ xp^T                          [512, T]  (q,k of 4 heads, transposed)
  v    = xp^T.T @ w_v_local^T                           [T, 256]  (natural layout, + ones col)
  per head h, per 512-wide query block: scores^T = k_h^T.T @ q_h^T (causal,
  lower j-tiles only), p^T = exp(scores^T / 8 - 3) (masked diag),
  y^T = [v|1]^T.T @ p^T accumulated over j-tiles -> row 64 of y^T is the
  softmax denominator (the -3 bias cancels in the ratio; it keeps f16 safe).
  y_cat^T[c_local, t] = y^T / denom; out_partial = y_cat^T.T @ w_proj_local^T  [T, C]
Host sums the 4 partial outputs per batch (unshard of the row-sharded proj).

All matmuls run in float16 (full-rate PE mode, fp32 PSUM accumulation).
"""

import numpy as np

B, T, C, H = 2, 2048, 1024, 16
NCORES = 8
GROUPS = 4            # head-groups across cores (tensor parallel)
HL = H // GROUPS      # heads per core = 4
D = C // H            # 64
CL = HL * D           # 256 local channels
KC = C // 128         # 8 contraction tiles over C
JG = 2                # j-tiles per scores psum tile (exp batch)
N_WARM = 28           # PE warmup matmuls (HAM un-throttle during initial DMA)

_PROG_CACHE = {}


def _build_program(t_len=T, debug_taps=False):
    from contextlib import ExitStack

    import concourse.tile as tile
    from concourse import bacc, mybir
    from concourse.masks import make_upper_triangular

    f32 = mybir.dt.float32
    f16 = mybir.dt.float16

    nt = t_len // 512     # 512-wide t chunks
    mt_n = t_len // 128   # 128-wide t tiles

    nc = bacc.Bacc("TRN2", target_bir_lowering=False, debug=False,
                   num_devices=NCORES)

    # chunk-major, partition-tiled host layouts -> fully contiguous DMAs
    x_r = nc.dram_tensor("x_t", [t_len // 512, 128, KC, 512], f16,
                         kind="ExternalInput").ap()
    pe_r = nc.dram_tensor("pe_t", [t_len // 512, 128, KC, 512], f16,
                          kind="ExternalInput").ap()
    wqk_r = nc.dram_tensor("w_qk_t", [128, KC, 2 * CL], f16,
                           kind="ExternalInput").ap()
    wv_r = nc.dram_tensor("w_v_t", [128, KC, CL], f16,
                          kind="ExternalInput").ap()
    wproj_r = nc.dram_tensor("w_proj_t", [128, CL // 128, C], f16,
                             kind="ExternalInput").ap()
    out = nc.dram_tensor("out", [t_len, C], f32, kind="ExternalOutput").ap()

    dbg = {}
    if debug_taps:
        nb_ = t_len // 512
        dbg["qk"] = nc.dram_tensor("dbg_qk", [4, 128, t_len], f16,
                                   kind="ExternalOutput").ap()
        dbg["v"] = nc.dram_tensor("dbg_v", [mt_n, 128, HL * (D + 1)], f16,
                                  kind="ExternalOutput").ap()
        dbg["ycat"] = nc.dram_tensor("dbg_ycat", [CL // 128, 128, t_len], f16,
                                     kind="ExternalOutput").ap()
        dbg["sum"] = nc.dram_tensor("dbg_sum", [nb_, HL, 512], f32,
                                    kind="ExternalOutput").ap()
        dbg["recip"] = nc.dram_tensor("dbg_recip", [nb_, HL, 512], f32,
                                      kind="ExternalOutput").ap()
        dbg["rb"] = nc.dram_tensor("dbg_rb", [nb_, HL, 64, 512], f32,
                                   kind="ExternalOutput").ap()

    with tile.TileContext(nc) as tc:
        with ExitStack() as ctx:
            const_p = ctx.enter_context(tc.tile_pool(name="const", bufs=1))
            w_p = ctx.enter_context(tc.tile_pool(name="weights", bufs=1))
            act_p = ctx.enter_context(tc.tile_pool(name="acts", bufs=1))
            work_p = ctx.enter_context(tc.tile_pool(name="work", bufs=2))
            ps_p = ctx.enter_context(tc.tile_pool(name="ps", bufs=2, space="PSUM"))

            mask_ut = const_p.tile([128, 128], f16, tag="mask")
            make_upper_triangular(nc, mask_ut[:], val=1.0, diag=True)
            ones_f32 = const_p.tile([128, HL], f32, tag="ones")
            nc.vector.memset(ones_f32[:], 1.0)
            expbias = const_p.tile([128, 1], f32, tag="expbias")
            nc.vector.memset(expbias[:], -3.0)
            wrm = const_p.tile([128, 512], f16, tag="wrm")
            nc.vector.memset(wrm[:], 0.0)

            # PE warmup: dense dummy f16 matmuls while the first DMAs land,
            # so the HAM clock gate is at 8/8 when real matmuls start.
            for _ in range(N_WARM):
                wps = ps_p.tile([128, 512], f32, tag="mmps", name="wps")
                nc.tensor.matmul(wps[:], lhsT=wrm[:, 0:128], rhs=wrm[:],
                                 start=True, stop=True)

            wqk_sb = w_p.tile([128, KC, 2 * CL], f16, tag="wqk")
            wv_sb = w_p.tile([128, KC, CL], f16, tag="wv")
            wproj_sb = w_p.tile([128, CL // 128, C], f16, tag="wpj")

            # ---- persistent activations ----
            # qk^T rows: m0 = q heads 0,1; m1 = q heads 2,3; m2 = k h0,1; m3 = k h2,3
            qk_sb = [act_p.tile([128, t_len], f16, tag=f"qk{m}", name="qk")
                     for m in range(4)]
            # v tiles, per 128-t tile: 4 heads x (64 v cols + ones col)
            v_sb = [act_p.tile([128, HL * (D + 1)], f16, tag=f"v{m}", name="v")
                    for m in range(mt_n)]
            for m in range(mt_n):
                ones_col = v_sb[m].rearrange("p (h e) -> p h e", e=D + 1)[:, :, D:D + 1]
                nc.vector.tensor_copy(ones_col,
                                      ones_f32.rearrange("p (h o) -> p h o", o=1))
            # y_cat^T [256, T] as 2 tiles of 128 partitions
            ycat_sb = [act_p.tile([128, t_len], f16, tag=f"ycat{k}", name="ycat")
                       for k in range(CL // 128)]

            xp_of = {}

            def emit_chunk_dma(n):
                """One 1MiB DMA for x (sync ring) and pe (act ring) each —
                a single InstDMACopy spreads across all 16 SDMA engines."""
                x_s = work_p.tile([128, KC, 512], f16, tag="x", bufs=1, name="x_s")
                pe_s = work_p.tile([128, KC, 512], f16, tag="pe", bufs=1,
                                   name="pe_s")
                xp = work_p.tile([128, KC, 512], f16, tag="xp", name="xp")
                nc.sync.dma_start(out=x_s[:], in_=x_r[n])
                nc.scalar.dma_start(out=pe_s[:], in_=pe_r[n])
                for q in range(4):
                    sl = slice(2 * q, 2 * q + 2)
                    nc.vector.tensor_add(xp[:, sl, :], x_s[:, sl, :],
                                         pe_s[:, sl, :])
                xp_of[n] = xp

            def chunk_fillers(n):
                """qk / v m-tile units for chunk n, as weavable closures."""
                def qk_unit(i):
                    def go():
                        xp = xp_of[n]
                        ps = ps_p.tile([128, 512], f32, tag="mmps", name="qk_ps")
                        for ck in range(KC):
                            nc.tensor.matmul(
                                ps[:],
                                lhsT=wqk_sb[:, ck, i * 128:(i + 1) * 128],
                                rhs=xp[:, ck, :],
                                start=(ck == 0), stop=(ck == KC - 1))
                        nc.vector.tensor_copy(
                            qk_sb[i][:, n * 512:(n + 1) * 512], ps[:])
                    return go

                def v_unit(i):
                    def go():
                        xp = xp_of[n]
                        psv = ps_p.tile([128, CL], f32, tag="mmps", name="v_ps")
                        for ck in range(KC):
                            nc.tensor.matmul(
                                psv[:],
                                lhsT=xp[:, ck, i * 128:(i + 1) * 128],
                                rhs=wv_sb[:, ck, :],
                                start=(ck == 0), stop=(ck == KC - 1))
                        mt = 4 * n + i
                        nc.vector.tensor_copy(
                            v_sb[mt].rearrange("p (h e) -> p h e",
                                               e=D + 1)[:, :, 0:D],
                            psv.rearrange("p (h e) -> p h e", e=D))
                    return go
                units = []
                for i in range(4):
                    units.append(qk_unit(i))
                    units.append(v_unit(i))
                return units

            def emit_chunk_compute(n):
                for f in chunk_fillers(n):
                    f()

            def _emit_norm_head(c, h, recips4, ysbs):
                hb = (h % 2) * 64
                rbsrc = work_p.tile([1, 512], f32, tag="rbsrc", bufs=4,
                                    name="rbsrc")
                nc.vector.tensor_copy(rbsrc[:], recips4[32 * h:32 * h + 1, :])
                rb = work_p.tile([64, 512], f32, tag="rb", bufs=3, name="rb")
                nc.gpsimd.partition_broadcast(rb[:], rbsrc[:])
                if dbg:
                    nc.sync.dma_start(out=dbg["rb"][c, h], in_=rb[:])
                nc.vector.tensor_mul(
                    ycat_sb[h // 2][hb:hb + 64, c * 512:(c + 1) * 512],
                    ysbs[h][0:64, :], rb[:])

            def emit_attn_block(c, fillers=(), split_tail=False):
                """Attention for query block i in [512c, 512c+512), all heads.
                One filler (qk/v/proj m-tile) is woven in after each score/AV
                group so the in-order PE queue has work during exp latency."""
                fillers = list(fillers)
                njt = 4 * c + 4
                sums4 = work_p.tile([128, 512], f32, tag="sums4", name="sums4")
                nc.vector.memset(sums4[:], 1.0)
                ysbs = {}
                # heads processed in interleaved pairs: head h+1's scores fill
                # the PE while head h's exp runs (in-order engine queues)
                for hp in (0, 2):
                    yps_of, esb_of, offs_of = {}, {}, {}
                    for h in (hp, hp + 1):
                        yps_of[h] = ps_p.tile([65, 512], f32, tag="yps",
                                              name="yps")
                    for g0 in range(0, njt, JG):
                        jts = range(g0, min(g0 + JG, njt))
                        for h in (hp, hp + 1):
                            hb = (h % 2) * 64
                            q_tile = qk_sb[h // 2]
                            k_tile = qk_sb[2 + h // 2]
                            sps = ps_p.tile([128, JG * 512], f32, tag="sps",
                                            name="sps")
                            esb = work_p.tile([128, JG * 512], f16, tag="esb",
                                              bufs=4, name="esb")
                            offs = {}
                            cover_end = None
                            for jt in jts:
                                off = max(0, (jt - 4 * c)) * 128
                                offs[jt] = off
                                ls = (jt - g0) * 512 + off
                                width = 512 - off
                                if cover_end is not None and ls > cover_end:
                                    # dead gap between j-tile ranges: zero it so
                                    # the batched exp reads initialized psum
                                    nc.vector.memset(sps[:, cover_end:ls], 0.0)
                                cover_end = ls + width
                                nc.tensor.matmul(
                                    sps[:, ls:ls + width],
                                    lhsT=k_tile[hb:hb + 64,
                                                jt * 128:(jt + 1) * 128],
                                    rhs=q_tile[hb:hb + 64,
                                               c * 512 + off:(c + 1) * 512],
                                    start=True, stop=True)
                            gfirst = offs[jts[0]]
                            gend = (jts[-1] - g0) * 512 + 512
                            nc.scalar.activation(
                                esb[:, gfirst:gend], sps[:, gfirst:gend],
                                mybir.ActivationFunctionType.Exp, scale=0.125,
                                bias=expbias[:])
                            for jt in jts:
                                if jt >= 4 * c:  # diagonal tile: causal mask
                                    ls = (jt - g0) * 512 + offs[jt]
                                    nc.vector.tensor_mul(
                                        esb[:, ls:ls + 128], esb[:, ls:ls + 128],
                                        mask_ut[:])
                            esb_of[h], offs_of[h] = esb, offs
                        for h in (hp, hp + 1):
                            esb, offs = esb_of[h], offs_of[h]
                            for jt in jts:
                                off = offs[jt]
                                ls = (jt - g0) * 512 + off
                                nc.tensor.matmul(
                                    yps_of[h][:, off:512],
                                    lhsT=v_sb[jt][:, h * (D + 1):
                                                  (h + 1) * (D + 1)],
                                    rhs=esb[:, ls:ls + (512 - off)],
                                    start=(jt == 0), stop=(jt == njt - 1))
                        if fillers:
                            fillers.pop(0)()
                    for h in (hp, hp + 1):
                        ysb = work_p.tile([65, 512], f32, tag="ysb", bufs=6,
                                          name="ysb")
                        nc.vector.tensor_copy(ysb[:], yps_of[h][:])
                        nc.vector.tensor_copy(sums4[32 * h:32 * h + 1, :],
                                              ysb[64:65, :])
                        ysbs[h] = ysb
                    if split_tail:
                        rc = work_p.tile([128, 512], f32, tag="recips4",
                                         name="rc")
                        ln_t = work_p.tile([128, 512], f32, tag="lns",
                                           name="ln_t")
                        nc.scalar.activation(ln_t[:], sums4[:],
                                             mybir.ActivationFunctionType.Ln)
                        nc.scalar.activation(rc[:], ln_t[:],
                                             mybir.ActivationFunctionType.Exp,
                                             scale=-1.0)
                        for h in (hp, hp + 1):
                            _emit_norm_head(c, h, rc, ysbs)
                if split_tail:
                    for f in fillers:
                        f()
                    del fillers[:]
                    return
                recips4 = work_p.tile([128, 512], f32, tag="recips4",
                                      name="recips4")
                lns = work_p.tile([128, 512], f32, tag="lns", name="lns")
                nc.scalar.activation(lns[:], sums4[:],
                                     mybir.ActivationFunctionType.Ln)
                nc.scalar.activation(recips4[:], lns[:],
                                     mybir.ActivationFunctionType.Exp,
                                     scale=-1.0)
                if dbg:
                    for hh in range(HL):
                        nc.sync.dma_start(out=dbg["sum"][c, hh],
                                          in_=sums4[32 * hh:32 * hh + 1, :])
                        nc.sync.dma_start(out=dbg["recip"][c, hh],
                                          in_=recips4[32 * hh:32 * hh + 1, :])
                for f in fillers:
                    f()
                del fillers[:]
                for h in range(HL):
                    _emit_norm_head(c, h, recips4, ysbs)

            def proj_fillers(c):
                def unit(i):
                    def go():
                        _emit_proj_mt(c, i)
                    return go
                return [unit(i) for i in range(4)]

            def emit_proj_block(c, fast_tail=False):
                """out rows [512c, 512c+512)."""
                for i in range(4):
                    _emit_proj_mt(c, i, fast_tail)

            def _emit_proj_mt(c, i, fast_tail=False):
                    mt = 4 * c + i
                    osb = work_p.tile([128, C], f32, tag="osb", bufs=3, name="osb")
                    for n2 in range(C // 512):
                        ps = ps_p.tile([128, 512], f32, tag="mmps", name="proj_ps")
                        for kk in range(CL // 128):
                            nc.tensor.matmul(
                                ps[:],
                                lhsT=ycat_sb[kk][:, mt * 128:(mt + 1) * 128],
                                rhs=wproj_sb[:, kk, n2 * 512:(n2 + 1) * 512],
                                start=(kk == 0), stop=(kk == CL // 128 - 1))
                        if fast_tail and n2 % 2 == 0:
                            nc.scalar.copy(osb[:, n2 * 512:(n2 + 1) * 512], ps[:])
                        else:
                            nc.vector.tensor_copy(
                                osb[:, n2 * 512:(n2 + 1) * 512], ps[:])
                    if fast_tail:
                        nc.sync.dma_start(out=out[mt * 128:(mt + 1) * 128, 0:512],
                                          in_=osb[:, 0:512])
                        nc.scalar.dma_start(
                            out=out[mt * 128:(mt + 1) * 128, 512:C],
                            in_=osb[:, 512:C])
                    else:
                        nc.sync.dma_start(out=out[mt * 128:(mt + 1) * 128, :],
                                          in_=osb[:])

            # attn(c) needs qkv chunks <= c; proj(c) needs attn(c).  Chunk
            # n+1's DMA + add are emitted before attn(n) (not queued behind the
            # attention tail), and qkv(n+1)/proj(n-1) m-tiles are woven into
            # attn(n)'s groups as PE fillers for the exp latency.
            emit_chunk_dma(0)
            # weights after chunk-0 x/pe: halves on the two HWDGE rings
            nc.sync.dma_start(out=wqk_sb[:, 0:KC // 2, :],
                              in_=wqk_r[:, 0:KC // 2, :])
            nc.scalar.dma_start(out=wqk_sb[:, KC // 2:KC, :],
                              in_=wqk_r[:, KC // 2:KC, :])
            nc.scalar.dma_start(out=wv_sb[:], in_=wv_r[:])
            nc.scalar.dma_start(out=wproj_sb[:], in_=wproj_r[:])
            units0 = chunk_fillers(0)
            for u in units0[0::2]:      # qk units first (need only wqk)
                u()
            for u in units0[1::2]:      # v units after (wv arrives later)
                u()
            for n in range(1, nt):
                emit_chunk_dma(n)
                emit_attn_block(n - 1)
                emit_chunk_compute(n)
                if n >= 2:
                    emit_proj_block(n - 2)
            emit_attn_block(nt - 1, split_tail=True)
            if nt >= 2:
                emit_proj_block(nt - 2)
            emit_proj_block(nt - 1, fast_tail=True)

            if dbg:
                for m in range(4):
                    nc.sync.dma_start(out=dbg["qk"][m], in_=qk_sb[m][:])
                for m in range(mt_n):
                    nc.sync.dma_start(out=dbg["v"][m], in_=v_sb[m][:])
                for k in range(CL // 128):
                    nc.sync.dma_start(out=dbg["ycat"][k], in_=ycat_sb[k][:])

    nc.compile()
    return nc


def _shard_inputs(x, w_qkv, w_proj, pe, t_len=T):
    x = np.asarray(x, dtype=np.float32).astype(np.float16)
    w_qkv = np.asarray(w_qkv, dtype=np.float32).astype(np.float16)
    w_proj = np.asarray(w_proj, dtype=np.float32).astype(np.float16)
    pe = np.asarray(pe, dtype=np.float32).astype(np.float16)

    def chunk_major(a_t):      # [C, t] -> [nt, 128, KC, 512]
        return np.ascontiguousarray(
            a_t.reshape(KC, 128, t_len // 512, 512).transpose(2, 1, 0, 3))

    def part_tiled(w_t):       # [C_in, M] -> [128, C_in//128, M]
        return np.ascontiguousarray(
            w_t.reshape(-1, 128, w_t.shape[1]).transpose(1, 0, 2))

    pe_t = chunk_major(pe[:t_len].T)
    x_ts = [chunk_major(x[b, :t_len].T) for b in range(x.shape[0])]
    in_maps = []
    for core in range(NCORES):
        b, g = core // GROUPS, core % GROUPS
        rows_q = w_qkv[g * CL:(g + 1) * CL]
        rows_k = w_qkv[C + g * CL:C + (g + 1) * CL]
        rows_v = w_qkv[2 * C + g * CL:2 * C + (g + 1) * CL]
        in_maps.append({
            "x_t": x_ts[b],
            "pe_t": pe_t,
            "w_qk_t": part_tiled(np.concatenate([rows_q, rows_k], axis=0).T.copy()),
            "w_v_t": part_tiled(rows_v.T.copy()),
            "w_proj_t": part_tiled(w_proj[:, g * CL:(g + 1) * CL].T.copy()),
        })
    return in_maps


_RUN_KWARGS = {}       # test-harness hook (e.g. trace=True); empty when graded
_LAST_RESULT = None


def kernel(x, w_qkv, w_proj, pe):
    global _LAST_RESULT
    from concourse import bass_utils

    if T not in _PROG_CACHE:
        _PROG_CACHE[T] = _build_program(T)
    nc = _PROG_CACHE[T]

    in_maps = _shard_inputs(x, w_qkv, w_proj, pe)
    res = bass_utils.run_bass_kernel_spmd(nc, in_maps, core_ids=list(range(NCORES)),
                                          **_RUN_KWARGS)
    _LAST_RESULT = res

    out = np.zeros((B, T, C), dtype=np.float32)
    for core in range(NCORES):
        out[core // GROUPS] += res.results[core]["out"]
    return out
